# revision 21
# baseline (speedup 1.0000x reference)
"""Trainium2 distributed kernel for nn_Assembler (dense transformer assembler).

Sharding: 8 cores = 2 batch groups x 4 ranks. Within a group:
- cross/self attention sequence-parallel (q rows SL=256/core), QKVO replicated
- FFN tensor-parallel (FF/4 per core) with one f32 AllReduce per layer
- lm_head vocab-sharded (8000/core), agreement net output-sharded
- attn_weights: unnormalized exp-scores + reciprocal rowsums dumped, host normalizes
All matmuls bf16 (f32 PSUM accumulate); residual stream / norms / softmax sums f32.
"""
import numpy as np
import ml_dtypes
from contextlib import ExitStack

import concourse.bass as bass
import concourse.bacc as bacc
import concourse.tile as tile
from concourse import mybir
from concourse.bass_utils import run_bass_kernel_spmd
from concourse.masks import make_identity

f32 = mybir.dt.float32
f32r = mybir.dt.float32r
bf16 = mybir.dt.bfloat16
AF = mybir.ActivationFunctionType
ALU = mybir.AluOpType

D = 1024; H = 8; HD = 128; NR = 4; B = 2; SQ = 1024; SKV = 512
VOCAB = 32000; NL = 6; FF = 4096
R = 4; SL = SQ // R; VS = VOCAB // R; FS = FF // R; NKV = NR * SKV
RG = [[0, 1, 2, 3], [4, 5, 6, 7]]
EPS = 1e-6
P = 128
ISQ = 1.0 / np.sqrt(HD)

_uid = [0]
def uid():
    _uid[0] += 1
    return _uid[0]

def build(stage=None):
    import os
    stage = stage or os.environ.get("CK_STAGE", "none")
    nc = bacc.Bacc("TRN2", target_bir_lowering=False, debug=False, num_devices=8)

    def din(name, shape, dt=f32):
        return nc.dram_tensor(name, shape, dt, kind="ExternalInput").ap()

    # ---- inputs (per-core, host prepped) ----
    xT0 = din("xT0", [D, SL])                 # query hidden local rows, feature-major
    kvT = din("kvT", [D, SKV], bf16)          # weighted kv (response r of group), feature-major
    sscale = din("sscale", [P, 1])            # 1/(SKV*conf)
    ca_wqT = din("ca_wqT", [2, D, D], bf16)
    ca_wkT = din("ca_wkT", [2, D, D], bf16)
    ca_wvT = din("ca_wvT", [2, D, D], bf16)
    ca_woT = din("ca_woT", [2, D, D], bf16)
    ca_bq = din("ca_bq", [2, P, 8])
    ca_bk = din("ca_bk", [2, P, 8])
    ca_bv = din("ca_bv", [2, 1, D], bf16)     # row layout for K=1 matmul
    ca_ob = din("ca_ob", [2, P, 8])
    cnw = din("cnw", [2, P, 8])
    agw1T = din("agw1T", [NR * D, 512], bf16)
    agb1 = din("agb1", [P, 4])
    agw2T = din("agw2T", [2 * D, 256], bf16)
    agb2 = din("agb2", [P, 2])
    agnw = din("agnw", [P, 8])
    gn1 = din("gn1", [NL, P, 8])
    gn2 = din("gn2", [NL, P, 8])
    gwqT = din("gwqT", [NL, D, D], bf16)
    gwkT = din("gwkT", [NL, D, D], bf16)
    gwvT = din("gwvT", [NL, D, D], bf16)
    gwoT = din("gwoT", [NL, D, D], bf16)
    gwgT = din("gwgT", [NL, D, FF], bf16)
    gwuT = din("gwuT", [NL, D, FF], bf16)
    gwdT = din("gwdT", [NL, FF, D], bf16)
    gnw = din("gnw", [P, 8])
    cosT = din("cosT", [P, SL])               # cos table, local rows, [HD, SL]
    sinTs = din("sinTs", [P, SL])             # sign-folded sin table
    lmT = din("lmT", [D, VS], bf16)
    mw1T = din("mw1T", [D, 256], bf16)
    mb1 = din("mb1", [P, 2])
    mw2T = din("mw2T", [256, 3], bf16)
    mb2 = din("mb2", [3, 1])

    # ---- outputs ----
    logits = nc.dram_tensor("logits", [SQ, VS], f32, kind="ExternalOutput").ap()
    attnu = nc.dram_tensor("attnu", [H, NKV, SL], bf16, kind="ExternalOutput").ap()
    rrec = nc.dram_tensor("rrec", [H, SL], f32, kind="ExternalOutput").ap()
    meta = nc.dram_tensor("meta", [3, 1], f32, kind="ExternalOutput").ap()
    xdump = nc.dram_tensor("xdump", [D, SL], f32, kind="ExternalOutput").ap()

    with tile.TileContext(nc) as tc, ExitStack() as ctx:
        const = ctx.enter_context(tc.tile_pool(name="const", bufs=1))
        wpool = ctx.enter_context(tc.tile_pool(name="wpool", bufs=11))
        apool = ctx.enter_context(tc.tile_pool(name="apool", bufs=2))
        xpool = ctx.enter_context(tc.tile_pool(name="xpool", bufs=1))
        upool = ctx.enter_context(tc.tile_pool(name="upool", bufs=2))
        spool = ctx.enter_context(tc.tile_pool(name="spool", bufs=3))
        psum = ctx.enter_context(tc.tile_pool(name="psum", bufs=8, space="PSUM"))
        dram = ctx.enter_context(tc.tile_pool(name="dram", bufs=2, space="DRAM"))
        kv_ctx = ExitStack()
        kvpool = kv_ctx.enter_context(tc.tile_pool(name="kvpool", bufs=1))
        cross_ctx = ExitStack()
        ckpool = cross_ctx.enter_context(tc.tile_pool(name="ckpool", bufs=1))
        cvpool = cross_ctx.enter_context(tc.tile_pool(name="cvpool", bufs=1))

        def ptile(n, dt=f32, p=P):
            t = psum.tile([p, n], dt, tag="ps", name=f"ps{uid()}")
            return t

        def dump_x(tag):
            if stage == tag:
                for m in range(8):
                    nc.sync.dma_start(xdump[m * P:(m + 1) * P, :],
                                      x[:, m * SL:(m + 1) * SL])

        ones_col = const.tile([P, 1], f32)
        nc.vector.memset(ones_col, 1.0)
        ones_row = const.tile([1, P], f32)
        nc.vector.memset(ones_row, 1.0)
        ones_bf = const.tile([P, 1], bf16)
        nc.vector.memset(ones_bf, 1.0)
        ones_bf_row = const.tile([1, P], bf16)
        nc.vector.memset(ones_bf_row, 1.0)
        ident_bf = const.tile([P, P], bf16)
        make_identity(nc, ident_bf)
        eps_sb = const.tile([P, 1], f32)
        nc.vector.memset(eps_sb, EPS)

        # rope tables replicated x8 heads, bf16
        cos8 = const.tile([P, H * SL], bf16)
        sin8 = const.tile([P, H * SL], bf16)
        cmini = spool.tile([P, SL], f32, tag="sm", bufs=3, name=f"cm{uid()}")
        nc.sync.dma_start(cmini, cosT)
        smini = spool.tile([P, SL], f32, tag="sm", bufs=3, name=f"sm{uid()}")
        nc.sync.dma_start(smini, sinTs)
        for j in range(H):
            nc.vector.tensor_copy(cos8[:, j * SL:(j + 1) * SL], cmini)
            nc.vector.tensor_copy(sin8[:, j * SL:(j + 1) * SL], smini)

        # persistent residual stream, feature-major [feat_chunk m -> cols m*SL..]
        x = xpool.tile([P, 8 * SL], f32)
        for m in range(8):
            nc.sync.dma_start(x[:, m * SL:(m + 1) * SL], xT0[m * P:(m + 1) * P, :])
        # cross kv, feature-major bf16
        kv_sb = kvpool.tile([P, 8 * SKV], bf16, tag="kvst")
        for t in range(8):
            nc.sync.dma_start(kv_sb[:, t * SKV:(t + 1) * SKV], kvT[t * P:(t + 1) * P, :])

        def load_wT(src2d, kdim, ncols, tagn="wmat"):
            tiles = []
            for k in range(kdim // P):
                wt = wpool.tile([P, ncols], bf16, tag=tagn, name=f"wt{uid()}")
                nc.sync.dma_start(wt, src2d[k * P:(k + 1) * P, :])
                tiles.append(wt)
            return tiles

        def rms_norm(w_sb):
            """x [P,8*SL] f32 -> returns xn bf16 [P,8*SL]; w_sb [P,8]."""
            ss = psum.tile([1, SL], f32, tag="ps", name=f"ss{uid()}")
            for m in range(8):
                sq = apool.tile([P, SL], f32, tag="sq", bufs=3, name=f"sq{uid()}")
                nc.scalar.square(sq, x[:, m * SL:(m + 1) * SL])
                nc.tensor.matmul(ss, ones_col, sq, start=(m == 0), stop=(m == 7))
            sv = spool.tile([1, SL], f32, tag="sm", bufs=3, name=f"sv{uid()}")
            nc.scalar.activation(sv, ss, AF.Sqrt, bias=eps_sb[:1], scale=1.0 / D)
            rinv = spool.tile([1, SL], f32, tag="sm", bufs=3, name=f"ri{uid()}")
            nc.vector.reciprocal(rinv, sv)
            psb = ptile(SL)
            nc.tensor.matmul(psb, ones_row, rinv, start=True, stop=True)
            bc = apool.tile([P, SL], f32, tag="bc", bufs=2, name=f"bc{uid()}")
            nc.vector.tensor_copy(bc, psb)
            xn = apool.tile([P, 8 * SL], bf16, tag="xn", bufs=1, name=f"xn{uid()}")
            for m in range(8):
                nc.vector.scalar_tensor_tensor(
                    xn[:, m * SL:(m + 1) * SL], x[:, m * SL:(m + 1) * SL],
                    w_sb[:, m:m + 1], bc, ALU.mult, ALU.mult)
            return xn

        def proj_fm(wts, rhs_slices, N, out_cb):
            """out feature-major: for m: psum [P,N] = sum_k wts[k][:,m*P:+P].T @ rhs_slices[k]"""
            K = len(wts)
            for m in range(8):
                ps = ptile(N)
                for k in range(K):
                    nc.tensor.matmul(ps, wts[k][:, m * P:(m + 1) * P], rhs_slices[k],
                                     start=(k == 0), stop=(k == K - 1))
                out_cb(m, ps)

        def rope(src_bf):
            """src [P, 8*SL] bf16 -> rotated+combined new tile (same tag space)."""
            rot = apool.tile([P, 8 * SL], bf16, tag="qr", bufs=2, name=f"rot{uid()}")
            nc.sync.dma_start(rot[0:64, :], src_bf[64:128, :])
            nc.sync.dma_start(rot[64:128, :], src_bf[0:64, :])
            t1 = apool.tile([P, 8 * SL], bf16, tag="qr", bufs=2, name=f"t1{uid()}")
            nc.vector.tensor_mul(t1, src_bf, cos8)
            out = apool.tile([P, 8 * SL], bf16, tag="qk", bufs=3, name=f"rp{uid()}")
            nc.vector.tensor_mul(out, rot, sin8)
            nc.vector.tensor_add(out, out, t1)
            return out

        def allgather(bounce, out_shape, dt, tagn):
            out = dram.tile(out_shape, dt, tag=tagn + "o", name=f"{tagn}o{uid()}")
            nc.gpsimd.collective_compute(
                "AllGather", ALU.bypass, replica_groups=RG,
                ins=[bounce.opt()], outs=[out.opt()])
            return out

        def attention(q_sb, k_sb, v_sb, nchunks, CS, oT, dump):
            cpr = CS // P
            for h in range(H):
                u = upool.tile([P, nchunks * SL], bf16, tag="u", name=f"u{uid()}")
                for c in range(nchunks):
                    ps_s = ptile(SL)
                    koff = ((c // cpr) * 8 + h) * CS + (c % cpr) * P
                    nc.tensor.matmul(ps_s, k_sb[:, koff:koff + P],
                                     q_sb[:, h * SL:(h + 1) * SL], start=True, stop=True)
                    nc.scalar.activation(u[:, c * SL:(c + 1) * SL], ps_s, AF.Exp, scale=ISQ)
                if dump:
                    nc.sync.dma_start(
                        attnu[h].rearrange("(c p) q -> p c q", p=P),
                        u.rearrange("p (c q) -> p c q", c=nchunks))
                for qh in range(2):
                    ps_o = ptile(HD)
                    ps_r = ptile(1)
                    for c in range(nchunks):
                        lhs = u[:, c * SL + qh * P: c * SL + qh * P + P]
                        nc.tensor.matmul(ps_o, lhs, v_sb[:, c * D + h * HD: c * D + (h + 1) * HD],
                                         start=(c == 0), stop=(c == nchunks - 1))
                        nc.tensor.matmul(ps_r, lhs, ones_bf,
                                         start=(c == 0), stop=(c == nchunks - 1))
                    rr = spool.tile([P, 1], f32, tag="rr", bufs=8, name=f"rr{uid()}")
                    nc.vector.reciprocal(rr, ps_r)
                    if dump:
                        nc.sync.dma_start(rrec[h:h + 1, qh * P:(qh + 1) * P], rr)
                    on = apool.tile([P, HD], bf16, tag="on", name=f"on{uid()}")
                    nc.vector.tensor_scalar_mul(on, ps_o, rr)
                    ps_t = psum.tile([P, HD], bf16, tag="ps", name=f"pst{uid()}")
                    nc.tensor.transpose(ps_t, on, ident_bf)
                    nc.vector.tensor_copy(oT[:, h * SL + qh * P: h * SL + qh * P + P], ps_t)

        # ================= cross-attention blocks =================
        for i in range(2):
            cnw_sb = spool.tile([P, 8], f32, tag="nw", bufs=12, name=f"cnw{uid()}")
            nc.sync.dma_start(cnw_sb, cnw[i])
            bq_sb = spool.tile([P, 8], f32, tag="nw", bufs=12, name=f"bq{uid()}")
            nc.sync.dma_start(bq_sb, ca_bq[i])
            bk_sb = spool.tile([P, 8], f32, tag="nw", bufs=12, name=f"bk{uid()}")
            nc.sync.dma_start(bk_sb, ca_bk[i])
            bv_sb = spool.tile([1, D], bf16, tag="bv", name=f"bv{uid()}")
            nc.sync.dma_start(bv_sb, ca_bv[i])
            ob_sb = spool.tile([P, 8], f32, tag="nw", bufs=12, name=f"ob{uid()}")
            nc.sync.dma_start(ob_sb, ca_ob[i])

            xn = rms_norm(cnw_sb)
            xn_sl = [xn[:, k * SL:(k + 1) * SL] for k in range(8)]
            kv_sl = [kv_sb[:, k * SKV:(k + 1) * SKV] for k in range(8)]

            # q projection + bias -> bf16 feature-major
            q_sb = apool.tile([P, 8 * SL], bf16, tag="qk", bufs=3, name=f"q{uid()}")
            wq = load_wT(ca_wqT[i], D, D)
            proj_fm(wq, xn_sl, SL,
                    lambda m, ps: nc.vector.tensor_scalar_add(
                        q_sb[:, m * SL:(m + 1) * SL], ps, bq_sb[:, m:m + 1]))
            # k projection + bias -> bounce -> AG
            kl = apool.tile([P, 8 * SKV], bf16, tag="qk", bufs=3, name=f"kl{uid()}")
            wk = load_wT(ca_wkT[i], D, D)
            proj_fm(wk, kv_sl, SKV,
                    lambda m, ps: nc.vector.tensor_scalar_add(
                        kl[:, m * SKV:(m + 1) * SKV], ps, bk_sb[:, m:m + 1]))
            kb = dram.tile([D, SKV], bf16, tag="kb", name=f"kb{uid()}")
            for m in range(8):
                nc.gpsimd.dma_start(kb[m * P:(m + 1) * P, :], kl[:, m * SKV:(m + 1) * SKV])
            agk = allgather(kb, [R * D, SKV], bf16, "agk")
            # v projection (seq-major) + bias via K=1 matmul -> bounce -> AG
            wv = load_wT(ca_wvT[i], D, D)
            vb = dram.tile([SKV, D], bf16, tag="vb", name=f"vb{uid()}")
            for m in range(4):
                vl = apool.tile([P, D], bf16, tag="vl", name=f"vl{uid()}")
                for n in range(2):
                    ps = ptile(512)
                    for k in range(8):
                        nc.tensor.matmul(ps, kv_sb[:, k * SKV + m * P: k * SKV + m * P + P],
                                         wv[k][:, n * 512:(n + 1) * 512],
                                         start=(k == 0), stop=False)
                    nc.tensor.matmul(ps, ones_bf_row,
                                     bv_sb[:, n * 512:(n + 1) * 512], start=False, stop=True)
                    nc.vector.tensor_copy(vl[:, n * 512:(n + 1) * 512], ps)
                nc.gpsimd.dma_start(vb[m * P:(m + 1) * P, :], vl)
            agv = allgather(vb, [NKV, D], bf16, "agv")

            # stage k,v
            k_sb = ckpool.tile([P, 32 * SKV], bf16, tag="ckst", name=f"kst{uid()}")
            for t in range(32):
                nc.sync.dma_start(k_sb[:, t * SKV:(t + 1) * SKV], agk[t * P:(t + 1) * P, :])
            v_sb = cvpool.tile([P, 16 * D], bf16, tag="cvst", name=f"vst{uid()}")
            for c in range(16):
                nc.sync.dma_start(v_sb[:, c * D:(c + 1) * D], agv[c * P:(c + 1) * P, :])

            oT = apool.tile([P, 8 * SL], bf16, tag="oT", bufs=1, name=f"oT{uid()}")
            attention(q_sb, k_sb, v_sb, 16, SKV, oT, dump=(i == 1))
            oT_sl = [oT[:, h * SL:(h + 1) * SL] for h in range(H)]
            wo = load_wT(ca_woT[i], D, D)
            proj_fm(wo, oT_sl, SL,
                    lambda m, ps: nc.vector.scalar_tensor_tensor(
                        x[:, m * SL:(m + 1) * SL], ps, ob_sb[:, m:m + 1],
                        x[:, m * SL:(m + 1) * SL], ALU.add, ALU.add))

        dump_x("cross")
        cross_ctx.close()
        # ================= agreement =================
        sscale_sb = spool.tile([P, 1], f32, tag="rr", bufs=8, name=f"ssc{uid()}")
        nc.sync.dma_start(sscale_sb, sscale)
        sloc = spool.tile([P, 8], f32, tag="nw", bufs=12, name=f"sloc{uid()}")
        for t in range(8):
            tmp = spool.tile([P, 1], f32, tag="rr", bufs=8, name=f"st{uid()}")
            nc.vector.reduce_sum(tmp, kv_sb[:, t * SKV:(t + 1) * SKV], axis=mybir.AxisListType.X)
            nc.vector.tensor_scalar_mul(sloc[:, t:t + 1], tmp, sscale_sb)
        sb_b = dram.tile([D, 1], f32, tag="sb", name=f"sb{uid()}")
        nc.gpsimd.dma_start(sb_b.rearrange("(t p) o -> p (t o)", p=P), sloc)
        ags = allgather(sb_b, [NR * D, 1], f32, "ags")
        s_all = spool.tile([P, 32], f32, tag="sall", name=f"sall{uid()}")
        nc.sync.dma_start(s_all, ags.rearrange("(c p) o -> p (c o)", p=P))
        s_bf = spool.tile([P, 32], bf16, tag="sallb", bufs=4, name=f"sbf{uid()}")
        nc.vector.tensor_copy(s_bf, s_all)

        agb1_sb = spool.tile([P, 4], f32, tag="nw", bufs=12, name=f"agb1{uid()}")
        nc.sync.dma_start(agb1_sb, agb1)
        hps = [ptile(1) for _ in range(4)]
        for k in range(32):
            w1t = wpool.tile([P, 512], bf16, tag="wmat", name=f"w1t{uid()}")
            nc.sync.dma_start(w1t, agw1T[k * P:(k + 1) * P, :])
            for m in range(4):
                nc.tensor.matmul(hps[m], w1t[:, m * P:(m + 1) * P], s_bf[:, k:k + 1],
                                 start=(k == 0), stop=(k == 31))
        hsh = spool.tile([P, 4], f32, tag="nw", bufs=12, name=f"hsh{uid()}")
        for m in range(4):
            nc.scalar.activation(hsh[:, m:m + 1], hps[m], AF.Gelu, bias=agb1_sb[:, m:m + 1])
        hb = dram.tile([512, 1], f32, tag="hb", name=f"hb{uid()}")
        nc.gpsimd.dma_start(hb.rearrange("(t p) o -> p (t o)", p=P), hsh)
        agh = allgather(hb, [2 * D, 1], f32, "agh")
        h_all = spool.tile([P, 16], f32, tag="sall", name=f"hall{uid()}")
        nc.sync.dma_start(h_all, agh.rearrange("(c p) o -> p (c o)", p=P))
        h_bf = spool.tile([P, 16], bf16, tag="sallb", bufs=4, name=f"hbf{uid()}")
        nc.vector.tensor_copy(h_bf, h_all)

        agb2_sb = spool.tile([P, 2], f32, tag="nw", bufs=12, name=f"agb2{uid()}")
        nc.sync.dma_start(agb2_sb, agb2)
        zps = [ptile(1) for _ in range(2)]
        for k in range(16):
            w2t = wpool.tile([P, 256], bf16, tag="wmat", name=f"w2t{uid()}")
            nc.sync.dma_start(w2t, agw2T[k * P:(k + 1) * P, :])
            for m in range(2):
                nc.tensor.matmul(zps[m], w2t[:, m * P:(m + 1) * P], h_bf[:, k:k + 1],
                                 start=(k == 0), stop=(k == 15))
        z_sb = spool.tile([P, 2], f32, tag="nw", bufs=12, name=f"zsb{uid()}")
        for m in range(2):
            nc.vector.tensor_scalar_add(z_sb[:, m:m + 1], zps[m], agb2_sb[:, m:m + 1])
        zb = dram.tile([256, 1], f32, tag="zb", name=f"zb{uid()}")
        nc.gpsimd.dma_start(zb.rearrange("(t p) o -> p (t o)", p=P), z_sb)
        agz = allgather(zb, [D, 1], f32, "agz")
        z_all = spool.tile([P, 8], f32, tag="nw", bufs=12, name=f"zall{uid()}")
        nc.sync.dma_start(z_all, agz.rearrange("(c p) o -> p (c o)", p=P))

        agnw_sb = spool.tile([P, 8], f32, tag="nw", bufs=12, name=f"agnw{uid()}")
        nc.sync.dma_start(agnw_sb, agnw)
        z2 = spool.tile([P, 8], f32, tag="nw", bufs=12, name=f"z2{uid()}")
        nc.scalar.square(z2, z_all)
        zr = spool.tile([P, 1], f32, tag="rr", bufs=8, name=f"zr{uid()}")
        nc.vector.reduce_sum(zr, z2, axis=mybir.AxisListType.X)
        ps1 = psum.tile([1, 1], f32, tag="ps", name=f"ps1{uid()}")
        nc.tensor.matmul(ps1, zr, ones_col, start=True, stop=True)
        sc = spool.tile([1, 1], f32, tag="rr", bufs=8, name=f"sc{uid()}")
        nc.scalar.activation(sc, ps1, AF.Sqrt, bias=eps_sb[:1], scale=1.0 / D)
        rv = spool.tile([1, 1], f32, tag="rr", bufs=8, name=f"rv{uid()}")
        nc.vector.reciprocal(rv, sc)
        psb2 = psum.tile([P, 1], f32, tag="ps", name=f"psb{uid()}")
        nc.tensor.matmul(psb2, ones_row, rv, start=True, stop=True)
        rvb = spool.tile([P, 1], f32, tag="rr", bufs=8, name=f"rvb{uid()}")
        nc.vector.tensor_copy(rvb, psb2)
        agr = spool.tile([P, 8], f32, tag="nw", bufs=12, name=f"agr{uid()}")
        nc.vector.scalar_tensor_tensor(agr, z_all, rvb, agnw_sb, ALU.mult, ALU.mult)
        for m in range(8):
            nc.vector.tensor_scalar_add(x[:, m * SL:(m + 1) * SL],
                                        x[:, m * SL:(m + 1) * SL], agr[:, m:m + 1])

        dump_x("agree")
        kv_ctx.close()
        gkpool = ctx.enter_context(tc.tile_pool(name="gkpool", bufs=1))
        gvpool = ctx.enter_context(tc.tile_pool(name="gvpool", bufs=1))
        gapool = ctx.enter_context(tc.tile_pool(name="gapool", bufs=1))
        # ================= generator layers =================
        for l in range(NL):
            n1_sb = spool.tile([P, 8], f32, tag="nw", bufs=12, name=f"n1{uid()}")
            nc.sync.dma_start(n1_sb, gn1[l])
            xn = rms_norm(n1_sb)
            xn_sl = [xn[:, k * SL:(k + 1) * SL] for k in range(8)]

            q0 = apool.tile([P, 8 * SL], bf16, tag="qk", bufs=3, name=f"q0{uid()}")
            wq = load_wT(gwqT[l], D, D)
            proj_fm(wq, xn_sl, SL,
                    lambda m, ps: nc.vector.tensor_copy(q0[:, m * SL:(m + 1) * SL], ps))
            q_sb = rope(q0)
            k0 = apool.tile([P, 8 * SL], bf16, tag="qk", bufs=3, name=f"k0{uid()}")
            wk = load_wT(gwkT[l], D, D)
            proj_fm(wk, xn_sl, SL,
                    lambda m, ps: nc.vector.tensor_copy(k0[:, m * SL:(m + 1) * SL], ps))
            kl = rope(k0)
            kb = dram.tile([D, SL], bf16, tag="kb", name=f"gkb{uid()}")
            for m in range(8):
                nc.gpsimd.dma_start(kb[m * P:(m + 1) * P, :], kl[:, m * SL:(m + 1) * SL])
            agk = allgather(kb, [R * D, SL], bf16, "agk")

            wv = load_wT(gwvT[l], D, D)
            vb = dram.tile([SL, D], bf16, tag="vb", name=f"gvb{uid()}")
            for m in range(2):
                vl = apool.tile([P, D], bf16, tag="vl", name=f"gvl{uid()}")
                for n in range(2):
                    ps = ptile(512)
                    for k in range(8):
                        nc.tensor.matmul(ps, xn[:, k * SL + m * P: k * SL + m * P + P],
                                         wv[k][:, n * 512:(n + 1) * 512],
                                         start=(k == 0), stop=(k == 7))
                    nc.vector.tensor_copy(vl[:, n * 512:(n + 1) * 512], ps)
                nc.gpsimd.dma_start(vb[m * P:(m + 1) * P, :], vl)
            agv = allgather(vb, [SQ, D], bf16, "agv")

            k_sb = gkpool.tile([P, 32 * SL], bf16, tag="kst", name=f"gkst{uid()}")
            for t in range(32):
                nc.sync.dma_start(k_sb[:, t * SL:(t + 1) * SL], agk[t * P:(t + 1) * P, :])
            v_sb = gvpool.tile([P, 8 * D], bf16, tag="vst", name=f"gvst{uid()}")
            for c in range(8):
                nc.sync.dma_start(v_sb[:, c * D:(c + 1) * D], agv[c * P:(c + 1) * P, :])

            oT = apool.tile([P, 8 * SL], bf16, tag="oT", bufs=1, name=f"goT{uid()}")
            attention(q_sb, k_sb, v_sb, 8, SL, oT, dump=False)
            oT_sl = [oT[:, h * SL:(h + 1) * SL] for h in range(H)]
            wo = load_wT(gwoT[l], D, D)
            proj_fm(wo, oT_sl, SL,
                    lambda m, ps: nc.vector.tensor_add(
                        x[:, m * SL:(m + 1) * SL], x[:, m * SL:(m + 1) * SL], ps))

            dump_x(f"a{l}")
            n2_sb = spool.tile([P, 8], f32, tag="nw", bufs=12, name=f"n2{uid()}")
            nc.sync.dma_start(n2_sb, gn2[l])
            xn2 = rms_norm(n2_sb)
            xn2_sl = [xn2[:, k * SL:(k + 1) * SL] for k in range(8)]

            act = gapool.tile([P, 32 * SL], bf16, tag="act", bufs=1, name=f"act{uid()}")
            for blk in range(4):
                wgb = load_wT(gwgT[l][:, blk * 1024:(blk + 1) * 1024], D, 1024)
                gsb = gapool.tile([P, 8 * SL], bf16, tag="gst", bufs=2, name=f"gs{uid()}")
                for m in range(8):
                    psg = ptile(SL)
                    for k in range(8):
                        nc.tensor.matmul(psg, wgb[k][:, m * P:(m + 1) * P], xn2_sl[k],
                                         start=(k == 0), stop=(k == 7))
                    nc.scalar.activation(gsb[:, m * SL:(m + 1) * SL], psg, AF.Silu)
                wub = load_wT(gwuT[l][:, blk * 1024:(blk + 1) * 1024], D, 1024)
                for m in range(8):
                    psu = ptile(SL)
                    for k in range(8):
                        nc.tensor.matmul(psu, wub[k][:, m * P:(m + 1) * P], xn2_sl[k],
                                         start=(k == 0), stop=(k == 7))
                    nc.vector.tensor_mul(act[:, (blk * 8 + m) * SL:(blk * 8 + m + 1) * SL],
                                         gsb[:, m * SL:(m + 1) * SL], psu)
            for kblk in range(4):
                wdb = load_wT(gwdT[l][kblk * 1024:(kblk + 1) * 1024, :], 1024, 1024)
                for m in range(8):
                    psd = ptile(SL)
                    for k in range(8):
                        nc.tensor.matmul(psd, wdb[k][:, m * P:(m + 1) * P],
                                         act[:, (kblk * 8 + k) * SL:(kblk * 8 + k + 1) * SL],
                                         start=(k == 0), stop=(k == 7))
                    nc.vector.tensor_add(x[:, m * SL:(m + 1) * SL],
                                         x[:, m * SL:(m + 1) * SL], psd)

            dump_x(f"f{l}")
        # ================= final norm + lm head + meta =================
        gnw_sb = spool.tile([P, 8], f32, tag="nw", bufs=12, name=f"gnw{uid()}")
        nc.sync.dma_start(gnw_sb, gnw)
        xnf = rms_norm(gnw_sb)
        xnb = dram.tile([D, SL], bf16, tag="xnb", name=f"xnb{uid()}")
        for m in range(8):
            nc.gpsimd.dma_start(xnb[m * P:(m + 1) * P, :], xnf[:, m * SL:(m + 1) * SL])
        agx = allgather(xnb, [R * D, SL], bf16, "agx")
        ax = gkpool.tile([P, 32 * SL], bf16, tag="kst", name=f"ax{uid()}")
        for t in range(32):
            nc.sync.dma_start(ax[:, t * SL:(t + 1) * SL], agx[t * P:(t + 1) * P, :])

        NCH = 16
        NW = VS // NCH  # 500
        for n in range(NCH):
            lw = []
            for k in range(8):
                lt = wpool.tile([P, NW], bf16, tag="wmat", name=f"lw{uid()}")
                nc.sync.dma_start(lt, lmT[k * P:(k + 1) * P, n * NW:(n + 1) * NW])
                lw.append(lt)
            for m in range(8):
                ps = ptile(NW)
                for k in range(8):
                    lhs = ax[:, ((m // 2) * 8 + k) * SL + (m % 2) * P:
                             ((m // 2) * 8 + k) * SL + (m % 2) * P + P]
                    nc.tensor.matmul(ps, lhs, lw[k], start=(k == 0), stop=(k == 7))
                lo = apool.tile([P, NW], f32, tag="lo", bufs=2, name=f"lo{uid()}")
                nc.vector.tensor_copy(lo, ps)
                nc.sync.dma_start(logits[m * P:(m + 1) * P, n * NW:(n + 1) * NW], lo)

        # pooled mean over full seq (normed x) -> meta
        pr = spool.tile([P, 32], f32, tag="sall", name=f"pr{uid()}")
        for t in range(32):
            nc.vector.reduce_sum(pr[:, t:t + 1], ax[:, t * SL:(t + 1) * SL],
                                 axis=mybir.AxisListType.X)
        pool_t = spool.tile([P, 8], f32, tag="nw", bufs=12, name=f"pool{uid()}")
        nc.vector.reduce_sum(pool_t, pr.rearrange("p (r f) -> p f r", f=8),
                             axis=mybir.AxisListType.X)
        nc.vector.tensor_scalar_mul(pool_t, pool_t, 1.0 / SQ)
        pool_bf = spool.tile([P, 8], bf16, tag="sallb", bufs=4, name=f"poolb{uid()}")
        nc.vector.tensor_copy(pool_bf, pool_t)

        mb1_sb = spool.tile([P, 2], f32, tag="nw", bufs=12, name=f"mb1{uid()}")
        nc.sync.dma_start(mb1_sb, mb1)
        mw1 = load_wT(mw1T, D, 256)
        mps = [ptile(1) for _ in range(2)]
        for k in range(8):
            for m in range(2):
                nc.tensor.matmul(mps[m], mw1[k][:, m * P:(m + 1) * P], pool_bf[:, k:k + 1],
                                 start=(k == 0), stop=(k == 7))
        h1 = spool.tile([P, 2], bf16, tag="sallb", bufs=4, name=f"h1{uid()}")
        for m in range(2):
            nc.scalar.activation(h1[:, m:m + 1], mps[m], AF.Gelu, bias=mb1_sb[:, m:m + 1])
        mw2 = load_wT(mw2T, 256, 3)
        ps3 = psum.tile([3, 1], f32, tag="ps", name=f"ps3{uid()}")
        for k in range(2):
            nc.tensor.matmul(ps3, mw2[k], h1[:, k:k + 1], start=(k == 0), stop=(k == 1))
        mb2_sb = spool.tile([3, 1], f32, tag="rr", bufs=8, name=f"mb2{uid()}")
        nc.sync.dma_start(mb2_sb, mb2)
        meta_sb = spool.tile([3, 1], f32, tag="rr", bufs=8, name=f"meta{uid()}")
        nc.vector.tensor_scalar_add(meta_sb, ps3, mb2_sb)
        nc.sync.dma_start(meta, meta_sb)

    nc.compile()
    return nc


# ======================= host side =======================

def _bf(a):
    return np.ascontiguousarray(a.astype(np.float32)).astype(ml_dtypes.bfloat16)


def _f(a):
    return np.ascontiguousarray(a.astype(np.float32))


def _pcol(vec, nch):
    """[nch*128] -> [128, nch] per-partition column layout"""
    return _f(np.asarray(vec).reshape(nch, P).T)


def _make_core_inputs(inp, g, r):
    rh = np.asarray(inp["rh_hiddens"], np.float32)
    conf = np.asarray(inp["rh_confidences"], np.float32)
    qh = np.asarray(inp["query_hidden"], np.float32)
    caw = np.asarray(inp["ca_in_w"], np.float32)
    cab = np.asarray(inp["ca_in_b"], np.float32)
    cow = np.asarray(inp["ca_out_w"], np.float32)
    cob = np.asarray(inp["ca_out_b"], np.float32)

    kv_w = rh[r, g] * conf[g, r]                       # [SKV, D]
    pos = np.arange(r * SL, (r + 1) * SL, dtype=np.float32)
    inv = 1.0 / (10000.0 ** (np.arange(0, HD, 2, dtype=np.float32) / HD))
    f = np.outer(pos, inv)                             # [SL, 64]
    cosT = np.concatenate([np.cos(f), np.cos(f)], axis=1).T   # [HD, SL]
    sinT = np.concatenate([-np.sin(f), np.sin(f)], axis=1).T  # sign-folded

    m = {
        "xT0": _f(qh[g, r * SL:(r + 1) * SL, :].T),
        "kvT": _bf(kv_w.T),
        "sscale": _f(np.full((P, 1), 1.0 / (SKV * conf[g, r]))),
        "ca_wqT": _bf(np.stack([caw[i][:D].T for i in range(2)])),
        "ca_wkT": _bf(np.stack([caw[i][D:2 * D].T for i in range(2)])),
        "ca_wvT": _bf(np.stack([caw[i][2 * D:].T for i in range(2)])),
        "ca_woT": _bf(np.stack([cow[i].T for i in range(2)])),
        "ca_bq": _f(np.stack([_pcol(cab[i][:D], 8) for i in range(2)])),
        "ca_bk": _f(np.stack([_pcol(cab[i][D:2 * D], 8) for i in range(2)])),
        "ca_bv": _bf(np.stack([cab[i][2 * D:].reshape(1, D) for i in range(2)])),
        "ca_ob": _f(np.stack([_pcol(cob[i], 8) for i in range(2)])),
        "cnw": _f(np.stack([_pcol(np.asarray(inp["cross_norm_w"])[i], 8) for i in range(2)])),
        "agw1T": _bf(np.asarray(inp["ag_w1"], np.float32).T[:, r * 512:(r + 1) * 512]),
        "agb1": _pcol(np.asarray(inp["ag_b1"])[r * 512:(r + 1) * 512], 4),
        "agw2T": _bf(np.asarray(inp["ag_w2"], np.float32).T[:, r * 256:(r + 1) * 256]),
        "agb2": _pcol(np.asarray(inp["ag_b2"])[r * 256:(r + 1) * 256], 2),
        "agnw": _pcol(np.asarray(inp["ag_norm_w"]), 8),
        "gn1": _f(np.stack([_pcol(np.asarray(inp["gen_n1"])[l], 8) for l in range(NL)])),
        "gn2": _f(np.stack([_pcol(np.asarray(inp["gen_n2"])[l], 8) for l in range(NL)])),
        "gwqT": _bf(np.asarray(inp["gen_wq"], np.float32).transpose(0, 2, 1)),
        "gwkT": _bf(np.asarray(inp["gen_wk"], np.float32).transpose(0, 2, 1)),
        "gwvT": _bf(np.asarray(inp["gen_wv"], np.float32).transpose(0, 2, 1)),
        "gwoT": _bf(np.asarray(inp["gen_wo"], np.float32).transpose(0, 2, 1)),
        "gwgT": _bf(np.asarray(inp["gen_wg"], np.float32).transpose(0, 2, 1)),
        "gwuT": _bf(np.asarray(inp["gen_wu"], np.float32).transpose(0, 2, 1)),
        "gwdT": _bf(np.asarray(inp["gen_wd"], np.float32).transpose(0, 2, 1)),
        "gnw": _pcol(np.asarray(inp["gen_norm_w"]), 8),
        "cosT": _f(cosT),
        "sinTs": _f(sinT),
        "lmT": _bf(np.asarray(inp["lm_head_w"], np.float32).T[:, r * VS:(r + 1) * VS]),
        "mw1T": _bf(np.asarray(inp["m_w1"], np.float32).T),
        "mb1": _pcol(np.asarray(inp["m_b1"]), 2),
        "mw2T": _bf(np.asarray(inp["m_w2"], np.float32).T),
        "mb2": _f(np.asarray(inp["m_b2"]).reshape(3, 1)),
    }
    return m


_BUILT = [None]


def _get_built():
    if _BUILT[0] is None:
        _BUILT[0] = build()
    return _BUILT[0]


def kernel(**inputs):
    nc = _get_built()
    in_maps = []
    for core in range(8):
        g, r = divmod(core, 4)
        in_maps.append(_make_core_inputs(inputs, g, r))
    res = run_bass_kernel_spmd(nc, in_maps, core_ids=list(range(8)))
    return _assemble(res.results)


def _assemble(results):
    logits = np.zeros((B, SQ, VOCAB), np.float32)
    attnw = np.zeros((B, SQ, NKV), np.float32)
    meta = np.zeros((B, 3), np.float32)
    for core in range(8):
        g, r = divmod(core, 4)
        o = results[core]
        logits[g, :, r * VS:(r + 1) * VS] = np.asarray(o["logits"], np.float32)
        u = np.asarray(o["attnu"]).astype(np.float32)      # [H, NKV, SL]
        rr = np.asarray(o["rrec"], np.float32)             # [H, SL] recip rowsums
        attnw[g, r * SL:(r + 1) * SL, :] = (u * rr[:, None, :]).mean(axis=0).T
        if r == 0:
            meta[g] = np.asarray(o["meta"], np.float32)[:, 0]
    return logits, meta, attnw
